# revision 16
# baseline (speedup 1.0000x reference)
"""GraphUnetNoPool (7-layer GCN U-net, no pooling) on 8 trn2 NeuronCores.

Math: gn = D^-1/2 (g+I) D^-1/2;  layer: h' = relu(gn @ h @ W.T + b)
Rewrite: u = dinv*h;  v = (g+I) @ u;  h' = relu((dinv*v) @ W.T + b)
  =>  per-core row-slab m:  v.T[d, m] = sum_k u[k, d] * A[k, m]  (A symmetric:
      column slab of A == transposed row slab, so lhsT = u natural layout and
      rhs = A[:, slab] streams naturally from DRAM rows).
Sharding: 1D row-parallel. Core c owns rows [c*S, (c+1)*S). Each layer ends
with an AllGather of u (split into bf16 hi+lo for full-precision bf16-pair
matmul: A exact in bf16 since entries are {0,1,2}).
"""

import numpy as np
from contextlib import ExitStack

import concourse.bass as bass
import concourse.tile as tile
from concourse import bacc, mybir
from concourse.bass_utils import run_bass_kernel_spmd
from concourse.masks import make_identity

F32 = mybir.dt.float32
BF16 = mybir.dt.bfloat16

N, D, C, L = 8192, 256, 8, 7
S = N // C            # 1024 rows per core
KC = N // 128         # 64 k-chunks
MQ = S // 128         # 8 m-chunks per slab
R_DEFAULT = 30        # resident A chunks in SBUF (rest streamed from DRAM)
USE_LO = True


def build_nc(n=N, d=D, c=C, r=R_DEFAULT, use_lo=USE_LO, n_layers=L):
    s = n // c
    kc = n // 128
    mq = s // 128
    r = min(r, kc)
    nmh = s // 512 if s >= 512 else 1   # moving halves of 512 (psum bank)
    mw = min(s, 512)                    # moving width
    dh_n = d // 128                     # d chunks (2 for d=256)
    assert d % 128 == 0 and s % 128 == 0 and n % 128 == 0

    nc = bacc.Bacc("TRN2", target_bir_lowering=False, debug=False, num_devices=c)

    a_dram = nc.dram_tensor("a_slab", [n, s], F32, kind="ExternalInput")
    u0_dram = nc.dram_tensor("u0", [n, 2 * d], BF16, kind="ExternalInput")
    h0s_dram = nc.dram_tensor("h0_slab", [s, d], F32, kind="ExternalInput")
    dslab_dram = nc.dram_tensor("dinv_slab", [128, mq], F32, kind="ExternalInput")
    dbc_dram = nc.dram_tensor("dinv_bcast", [128, s], F32, kind="ExternalInput")
    wt_dram = nc.dram_tensor("wt", [n_layers, d, d], F32, kind="ExternalInput")
    bias_dram = nc.dram_tensor("bias_t", [128, 2 * n_layers], F32, kind="ExternalInput")
    out_dram = nc.dram_tensor("out", [4, s, d], F32, kind="ExternalOutput")

    UGRP = 4                # u tiles split in groups for finer dma/matmul deps
    assert kc % UGRP == 0
    kg = kc // UGRP

    with ExitStack() as ctx:
        tc = ctx.enter_context(tile.TileContext(nc))
        dram = ctx.enter_context(tc.tile_pool(name="dram", bufs=1, space="DRAM"))
        res = ctx.enter_context(tc.tile_pool(name="res", bufs=1))
        stage = ctx.enter_context(tc.tile_pool(name="stage", bufs=2))
        astream = ctx.enter_context(tc.tile_pool(name="astream", bufs=3))
        wtp = ctx.enter_context(tc.tile_pool(name="wtp", bufs=2))
        work = ctx.enter_context(tc.tile_pool(name="work", bufs=2))
        slabp = ctx.enter_context(tc.tile_pool(name="slabp", bufs=2))
        pmm1 = ctx.enter_context(tc.tile_pool(name="pmm1", bufs=4, space="PSUM"))
        post = ctx.enter_context(tc.tile_pool(name="post", bufs=4, space="PSUM"))

        # ---- persistent DRAM scratch ----
        ag_in = dram.tile([s, 2 * d], BF16, name="ag_in")
        ag_outs = [
            dram.tile(
                [n, 2 * d], BF16, name=f"ag_out{i}", tag=f"ag_out{i}",
                addr_space="Shared",
            )
            for i in range(n_layers - 1)
        ]
        skip_dram = dram.tile([3, s, d], F32, name="skip_dram")
        n_spill = kc - r
        if n_spill:
            a_spill = dram.tile([n_spill * 128, s], BF16, name="a_spill")

        # ---- persistent SBUF ----
        a_sb = res.tile([128, max(r, 1), s], BF16, name="a_sb")
        u_hi = [res.tile([128, kg, d], BF16, name=f"u_hi{g}") for g in range(UGRP)]
        u_lo = (
            [res.tile([128, kg, d], BF16, name=f"u_lo{g}") for g in range(UGRP)]
            if use_lo
            else None
        )
        dinv_sb = res.tile([128, mq], F32, name="dinv_sb")
        dinv_bc = res.tile([128, s], F32, name="dinv_bc")
        bias_sb = res.tile([128, 2 * n_layers], F32, name="bias_sb")
        ident = res.tile([128, 128], F32, name="ident")

        make_identity(nc, ident)
        nc.sync.dma_start(out=dinv_sb, in_=dslab_dram[:, :])
        nc.sync.dma_start(out=dinv_bc, in_=dbc_dram[:, :])
        nc.sync.dma_start(out=bias_sb, in_=bias_dram[:, :])

        # ---- startup: load A column-slab, cast to bf16 (resident + spill) ----
        for k in range(kc):
            st = stage.tile([128, s], F32, name="st", tag="stage")
            nc.sync.dma_start(out=st, in_=a_dram[k * 128 : (k + 1) * 128, :])
            if k < r:
                nc.vector.tensor_copy(a_sb[:, k, :], st)
            else:
                sb16 = astream.tile([128, s], BF16, name="sb16", tag="astream")
                nc.vector.tensor_copy(sb16, st)
                nc.sync.dma_start(
                    out=a_spill[(k - r) * 128 : (k - r + 1) * 128, :], in_=sb16
                )

        relu = mybir.ActivationFunctionType.Relu
        skip_slot = {4: 2, 5: 1, 6: 0}  # up-layer l uses skip h_{...} slot

        for l in range(n_layers):
            # ---- Phase A: load U (layer 0: host-packed u0; else allgathered) ----
            src = u0_dram if l == 0 else ag_outs[l - 1]
            src_ap = src[:, :] if l == 0 else src
            src3 = src_ap.rearrange("(g k p) td -> g p k td", p=128, g=UGRP)
            for g in range(UGRP):
                nc.sync.dma_start(out=u_hi[g], in_=src3[g, :, :, 0:d])
                if use_lo:
                    nc.sync.dma_start(out=u_lo[g], in_=src3[g, :, :, d : 2 * d])

            # per-layer weight prefetch
            wt_t = wtp.tile([128, dh_n, d], F32, name="wt_t", tag="wt")
            nc.sync.dma_start(
                out=wt_t, in_=wt_dram[l].rearrange("(kc p) o -> p kc o", p=128)
            )

            # skip-connection preload for NEXT layer's input (scaled by dinv)
            nl = l + 1
            skip_sb = None
            if nl in skip_slot and nl < n_layers:
                skip_sb = slabp.tile([128, mq, d], F32, name="skip_sb", tag="skip")
                nc.sync.dma_start(
                    out=skip_sb,
                    in_=skip_dram[skip_slot[nl]].rearrange(
                        "(m p) d2 -> p m d2", p=128
                    ),
                )
                for m in range(mq):
                    nc.vector.tensor_scalar(
                        out=skip_sb[:, m, :],
                        in0=skip_sb[:, m, :],
                        scalar1=dinv_sb[:, m : m + 1],
                        scalar2=None,
                        op0=mybir.AluOpType.mult,
                    )

            # ---- Phase B: mm1  v.T[d, m] accumulate over k (bf16 hi+lo) ----
            psv = [
                [pmm1.tile([128, mw], F32, name="psv", tag="pmm1") for _ in range(nmh)]
                for _ in range(dh_n)
            ]
            srcs = [u_hi, u_lo] if use_lo else [u_hi]
            for k in range(kc):
                if k < r:
                    rhs = a_sb[:, k, :]
                else:
                    ast = astream.tile([128, s], BF16, name="ast", tag="astream")
                    nc.sync.dma_start(
                        out=ast, in_=a_spill[(k - r) * 128 : (k - r + 1) * 128, :]
                    )
                    rhs = ast
                for si, usrc in enumerate(srcs):
                    lt = usrc[k // kg][:, k % kg, :]
                    for dh in range(dh_n):
                        for mh in range(nmh):
                            nc.tensor.matmul(
                                psv[dh][mh],
                                lt[:, dh * 128 : (dh + 1) * 128],
                                rhs[:, mh * mw : (mh + 1) * mw],
                                start=(k == 0 and si == 0),
                                stop=(k == kc - 1 and si == len(srcs) - 1),
                            )

            # ---- Phase C: copy v.T to SBUF ----
            v_sb = [
                work.tile([128, s], F32, name="v_sb", tag="vsb") for _ in range(dh_n)
            ]
            for dh in range(dh_n):
                for mh in range(nmh):
                    nc.vector.tensor_copy(
                        v_sb[dh][:, mh * mw : (mh + 1) * mw], psv[dh][mh]
                    )

            # ---- Phase D: mm2 (fp32)  out.T = W @ v.T ; relu(+bias) ----
            pso = [
                [post.tile([128, mw], F32, name="pso", tag="post") for _ in range(nmh)]
                for _ in range(dh_n)
            ]
            for dho in range(dh_n):
                for kin in range(dh_n):
                    lt = wt_t[:, kin, dho * 128 : (dho + 1) * 128]
                    for mh in range(nmh):
                        nc.tensor.matmul(
                            pso[dho][mh],
                            lt,
                            v_sb[kin][:, mh * mw : (mh + 1) * mw],
                            start=(kin == 0),
                            stop=(kin == dh_n - 1),
                        )
            hT = [work.tile([128, s], F32, name="hT", tag="hT") for _ in range(dh_n)]
            for dho in range(dh_n):
                for mh in range(nmh):
                    sl_ = slice(mh * mw, (mh + 1) * mw)
                    nc.vector.tensor_mul(hT[dho][:, sl_], pso[dho][mh], dinv_bc[:, sl_])
                    nc.scalar.activation(
                        hT[dho][:, sl_],
                        hT[dho][:, sl_],
                        relu,
                        bias=bias_sb[:, 2 * l + dho : 2 * l + dho + 1],
                    )

            # ---- Phase E: transpose to natural, build next u slab, outputs ----
            is_out = l >= n_layers - 3  # layers 4,5,6 emit outputs 0,1,2
            save_skip = l <= 2
            h_nat = None
            if is_out or save_skip:
                h_nat = slabp.tile([128, mq, d], F32, name="h_nat", tag="hnat", bufs=1)
            if l == n_layers - 1:
                # reuse the (now idle) skip/us tags for the final-layer tiles
                h0s = slabp.tile([128, mq, d], F32, name="h0s", tag="skip")
                nc.sync.dma_start(
                    out=h0s, in_=h0s_dram[:, :].rearrange("(m p) d2 -> p m d2", p=128)
                )
                out3 = slabp.tile([128, mq, d], F32, name="out3", tag="us_hi", bufs=1)
            if l < n_layers - 1:
                us_hi = slabp.tile(
                    [128, mq, d], BF16, name="us_hi", tag="us_hi", bufs=1
                )
                if use_lo:
                    us_lo = slabp.tile(
                        [128, mq, d], BF16, name="us_lo", tag="us_lo", bufs=1
                    )

            for m in range(mq):
                tp = post.tile([128, d], F32, name="tp", tag="post")
                for dh in range(dh_n):
                    nc.tensor.transpose(
                        tp[:, dh * 128 : (dh + 1) * 128],
                        hT[dh][:, m * 128 : (m + 1) * 128],
                        ident,
                    )
                if l < n_layers - 1:
                    ufp = stage.tile([128, d], F32, name="ufp", tag="ufp")
                    dv = dinv_sb[:, m : m + 1]
                    if skip_sb is not None:
                        nc.vector.scalar_tensor_tensor(
                            out=ufp,
                            in0=tp,
                            scalar=dv,
                            in1=skip_sb[:, m, :],
                            op0=mybir.AluOpType.mult,
                            op1=mybir.AluOpType.add,
                        )
                    else:
                        nc.vector.tensor_scalar(
                            out=ufp,
                            in0=tp,
                            scalar1=dv,
                            scalar2=None,
                            op0=mybir.AluOpType.mult,
                        )
                    nc.vector.tensor_copy(us_hi[:, m, :], ufp)
                    if use_lo:
                        nc.vector.tensor_sub(us_lo[:, m, :], ufp, us_hi[:, m, :])
                if h_nat is not None:
                    nc.scalar.copy(h_nat[:, m, :], tp)
                if l == n_layers - 1:
                    nc.vector.tensor_add(out3[:, m, :], tp, h0s[:, m, :])

            # ---- Phase F: DMAs out + AllGather ----
            if save_skip:
                nc.sync.dma_start(
                    out=skip_dram[l].rearrange("(m p) d2 -> p m d2", p=128),
                    in_=h_nat,
                )
            if is_out:
                nc.sync.dma_start(
                    out=out_dram[l - (n_layers - 3)].rearrange(
                        "(m p) d2 -> p m d2", p=128
                    ),
                    in_=h_nat,
                )
            if l == n_layers - 1:
                nc.sync.dma_start(
                    out=out_dram[3].rearrange("(m p) d2 -> p m d2", p=128), in_=out3
                )
            if l < n_layers - 1:
                agv = ag_in.rearrange("(m p) td -> p m td", p=128)
                nc.sync.dma_start(out=agv[:, :, 0:d], in_=us_hi)
                if use_lo:
                    nc.sync.dma_start(out=agv[:, :, d : 2 * d], in_=us_lo)
                nc.gpsimd.collective_compute(
                    "AllGather",
                    mybir.AluOpType.bypass,
                    replica_groups=[list(range(c))],
                    ins=[ag_in.opt()],
                    outs=[ag_outs[l].opt()],
                )

    nc.compile()
    return nc


def prep_inputs(g, h, W_down, b_down, W_bottom, b_bottom, W_up, b_up, c=C):
    """Host-side sharding + layout prep. Returns per-core input maps."""
    n = g.shape[0]
    s = n // c
    d = h.shape[1]
    g = np.asarray(g, np.float32)
    h = np.asarray(h, np.float32)
    deg = g.sum(axis=1) + 1.0
    dinv = (1.0 / np.sqrt(deg)).astype(np.float32)

    u0 = (h * dinv[:, None]).astype(np.float32)
    u0_hi = u0.astype(np.float32).astype(ml_bf16)
    u0_lo = (u0 - u0_hi.astype(np.float32)).astype(ml_bf16)
    u0_packed = np.concatenate(
        [np.asarray(u0_hi), np.asarray(u0_lo)], axis=1
    )  # [n, 2d] bf16

    Ws = [W_down[0], W_down[1], W_down[2], W_bottom, W_up[0], W_up[1], W_up[2]]
    bs = [b_down[0], b_down[1], b_down[2], b_bottom, b_up[0], b_up[1], b_up[2]]
    wt = np.stack([np.ascontiguousarray(np.asarray(W, np.float32).T) for W in Ws])
    nl = len(Ws)
    bias_t = np.zeros((128, 2 * nl), np.float32)
    for li, b in enumerate(bs):
        b = np.asarray(b, np.float32)
        for dh in range(d // 128):
            bias_t[:, 2 * li + dh] = b[dh * 128 : (dh + 1) * 128]

    in_maps = []
    for ci in range(c):
        sl = slice(ci * s, (ci + 1) * s)
        a_slab = np.ascontiguousarray(g[:, sl])
        idx = np.arange(s)
        a_slab[ci * s + idx, idx] += 1.0  # fold self-loops into the slab
        dinv_slab = dinv[sl].reshape(s // 128, 128).T.copy()  # [128, mq]
        dinv_bcast = np.broadcast_to(dinv[sl][None, :], (128, s)).copy()
        in_maps.append(
            dict(
                a_slab=a_slab,
                u0=u0_packed,
                h0_slab=np.ascontiguousarray(h[sl]),
                dinv_slab=dinv_slab,
                dinv_bcast=dinv_bcast,
                wt=wt,
                bias_t=bias_t,
            )
        )
    return in_maps


try:
    import ml_dtypes

    ml_bf16 = ml_dtypes.bfloat16
except ImportError:  # pragma: no cover
    import jax.numpy as jnp

    ml_bf16 = jnp.bfloat16

_NC_CACHE = {}


def kernel(g, h, W_down, b_down, W_bottom, b_bottom, W_up, b_up):
    key = "full"
    if key not in _NC_CACHE:
        _NC_CACHE[key] = build_nc()
    nc = _NC_CACHE[key]
    in_maps = prep_inputs(g, h, W_down, b_down, W_bottom, b_bottom, W_up, b_up)
    res = run_bass_kernel_spmd(nc, in_maps, list(range(C)))
    outs = [np.asarray(r["out"]).reshape(4, S, D) for r in res.results]
    full = np.concatenate(outs, axis=1)  # [4, N, D]
    return full.astype(np.float32)


if __name__ == "__main__":
    import reference

    inputs = reference.setup_inputs()
    inputs = {k: np.asarray(v) for k, v in inputs.items()}
    out = kernel(**inputs)
    exp = np.asarray(reference.reference(**reference.setup_inputs()))
    err = np.abs(out - exp).max() / (np.abs(exp).max() + 1e-30)
    rel = np.linalg.norm(out - exp) / (np.linalg.norm(exp) + 1e-30)
    print("max-scaled err:", err, "rel l2:", rel)


# revision 21
# speedup vs baseline: 10.2860x; 10.2860x over previous
"""GraphUnetNoPool (7-layer GCN U-net, no pooling) on 8 trn2 NeuronCores.

Math: gn = D^-1/2 (g+I) D^-1/2;  layer: h' = relu(gn @ h @ W.T + b)
Rewrite: u = dinv*h;  v = (g+I) @ u;  h' = relu((dinv*v) @ W.T + b)
  =>  per-core row-slab m:  v.T[d, m] = sum_k u[k, d] * A[k, m]  (A symmetric:
      column slab of A == transposed row slab, so lhsT = u natural layout and
      rhs = A[:, slab] streams naturally from DRAM rows).
Sharding: 1D row-parallel. Core c owns rows [c*S, (c+1)*S). Each layer ends
with an AllGather of u (split into bf16 hi+lo for full-precision bf16-pair
matmul: A exact in bf16 since entries are {0,1,2}).
"""

import numpy as np
from contextlib import ExitStack

import concourse.bass as bass
import concourse.tile as tile
from concourse import bacc, mybir
from concourse.bass_utils import run_bass_kernel_spmd
from concourse.masks import make_identity

F32 = mybir.dt.float32
BF16 = mybir.dt.bfloat16

N, D, C, L = 8192, 256, 8, 7
S = N // C            # 1024 rows per core
KC = N // 128         # 64 k-chunks
MQ = S // 128         # 8 m-chunks per slab
R_DEFAULT = 30        # resident A chunks in SBUF (rest streamed from DRAM)
USE_LO = True


def build_nc(n=N, d=D, c=C, r=R_DEFAULT, use_lo=USE_LO, n_layers=L, repeat=1):
    s = n // c
    kc = n // 128
    mq = s // 128
    r = min(r, kc)
    nmh = s // 512 if s >= 512 else 1   # moving halves of 512 (psum bank)
    mw = min(s, 512)                    # moving width
    dh_n = d // 128                     # d chunks (2 for d=256)
    assert d % 128 == 0 and s % 128 == 0 and n % 128 == 0

    nc = bacc.Bacc("TRN2", target_bir_lowering=False, debug=False, num_devices=c)

    a_dram = nc.dram_tensor("a_slab", [n, s], F32, kind="ExternalInput")
    u0_dram = nc.dram_tensor("u0", [n, 2 * d], BF16, kind="ExternalInput")
    h0s_dram = nc.dram_tensor("h0_slab", [s, d], F32, kind="ExternalInput")
    dslab_dram = nc.dram_tensor("dinv_slab", [128, mq], F32, kind="ExternalInput")
    dbc_dram = nc.dram_tensor("dinv_bcast", [128, s], F32, kind="ExternalInput")
    wt_dram = nc.dram_tensor("wt", [n_layers, d, d], F32, kind="ExternalInput")
    bias_dram = nc.dram_tensor("bias_t", [128, 2 * n_layers], F32, kind="ExternalInput")
    out_dram = nc.dram_tensor("out", [4, s, d], F32, kind="ExternalOutput")

    UGRP = 4                # u tiles split in groups for finer dma/matmul deps
    assert kc % UGRP == 0
    kg = kc // UGRP

    with ExitStack() as ctx:
        tc = ctx.enter_context(tile.TileContext(nc))
        dram = ctx.enter_context(tc.tile_pool(name="dram", bufs=1, space="DRAM"))
        res = ctx.enter_context(tc.tile_pool(name="res", bufs=1))
        stage = ctx.enter_context(tc.tile_pool(name="stage", bufs=2))
        astream = ctx.enter_context(tc.tile_pool(name="astream", bufs=3))
        wtp = ctx.enter_context(tc.tile_pool(name="wtp", bufs=2))
        work = ctx.enter_context(tc.tile_pool(name="work", bufs=2))
        slabp = ctx.enter_context(tc.tile_pool(name="slabp", bufs=2))
        pmm1 = ctx.enter_context(tc.tile_pool(name="pmm1", bufs=4, space="PSUM"))
        post = ctx.enter_context(tc.tile_pool(name="post", bufs=4, space="PSUM"))

        # ---- persistent DRAM scratch ----
        ag_in = dram.tile([s, 2 * d], BF16, name="ag_in")
        ag_outs = [
            dram.tile(
                [n, 2 * d], BF16, name=f"ag_out{i}", tag=f"ag_out{i}",
                addr_space="Shared",
            )
            for i in range((n_layers - 1) * repeat)
        ]
        skip_dram = dram.tile([3, s, d], F32, name="skip_dram")
        n_spill = kc - r
        if n_spill:
            a_spill = dram.tile([n_spill * 128, s], BF16, name="a_spill")

        # ---- persistent SBUF ----
        a_sb = res.tile([128, max(r, 1), s], BF16, name="a_sb")
        u_hi = [res.tile([128, kg, d], BF16, name=f"u_hi{g}") for g in range(UGRP)]
        u_lo = (
            [res.tile([128, kg, d], BF16, name=f"u_lo{g}") for g in range(UGRP)]
            if use_lo
            else None
        )
        dinv_sb = res.tile([128, mq], F32, name="dinv_sb")
        dinv_bc = res.tile([128, s], F32, name="dinv_bc")
        bias_sb = res.tile([128, 2 * n_layers], F32, name="bias_sb")
        ident = res.tile([128, 128], F32, name="ident")

        make_identity(nc, ident)
        nc.sync.dma_start(out=dinv_sb, in_=dslab_dram[:, :])
        nc.sync.dma_start(out=dinv_bc, in_=dbc_dram[:, :])
        nc.sync.dma_start(out=bias_sb, in_=bias_dram[:, :])

        # ---- startup: load A column-slab, cast to bf16 (resident + spill) ----
        for k in range(kc):
            st = stage.tile([128, s], F32, name="st", tag="stage")
            nc.sync.dma_start(out=st, in_=a_dram[k * 128 : (k + 1) * 128, :])
            if k < r:
                nc.vector.tensor_copy(a_sb[:, k, :], st)
            else:
                sb16 = astream.tile([128, s], BF16, name="sb16", tag="astream")
                nc.vector.tensor_copy(sb16, st)
                nc.sync.dma_start(
                    out=a_spill[(k - r) * 128 : (k - r + 1) * 128, :], in_=sb16
                )

        relu = mybir.ActivationFunctionType.Relu
        skip_slot = {4: 2, 5: 1, 6: 0}  # up-layer l uses skip h_{...} slot

        for rep_l in range(n_layers * repeat):
            rep, l = divmod(rep_l, n_layers)
            # ---- Phase A: load U (layer 0: host-packed u0; else allgathered) ----
            src = u0_dram if l == 0 else ag_outs[rep * (n_layers - 1) + l - 1]
            src_ap = src[:, :] if l == 0 else src
            src3 = src_ap.rearrange("(g k p) td -> g p k td", p=128, g=UGRP)
            for g in range(UGRP):
                nc.sync.dma_start(out=u_hi[g], in_=src3[g, :, :, 0:d])
                if use_lo:
                    nc.sync.dma_start(out=u_lo[g], in_=src3[g, :, :, d : 2 * d])

            # per-layer weight prefetch
            wt_t = wtp.tile([128, dh_n, d], F32, name="wt_t", tag="wt")
            nc.sync.dma_start(
                out=wt_t, in_=wt_dram[l].rearrange("(kc p) o -> p kc o", p=128)
            )

            # skip-connection preload for NEXT layer's input (scaled by dinv)
            nl = l + 1
            skip_sb = None
            if nl in skip_slot and nl < n_layers:
                skip_sb = slabp.tile([128, mq, d], F32, name="skip_sb", tag="skip")
                nc.sync.dma_start(
                    out=skip_sb,
                    in_=skip_dram[skip_slot[nl]].rearrange(
                        "(m p) d2 -> p m d2", p=128
                    ),
                )
                for m in range(mq):
                    nc.vector.tensor_scalar(
                        out=skip_sb[:, m, :],
                        in0=skip_sb[:, m, :],
                        scalar1=dinv_sb[:, m : m + 1],
                        scalar2=None,
                        op0=mybir.AluOpType.mult,
                    )

            # ---- Phase B: mm1  v.T[d, m] accumulate over k (bf16 hi+lo) ----
            psv = [
                [pmm1.tile([128, mw], F32, name="psv", tag="pmm1") for _ in range(nmh)]
                for _ in range(dh_n)
            ]
            srcs = [u_hi, u_lo] if use_lo else [u_hi]
            for k in range(kc):
                if k < r:
                    rhs = a_sb[:, k, :]
                else:
                    ast = astream.tile([128, s], BF16, name="ast", tag="astream")
                    nc.sync.dma_start(
                        out=ast, in_=a_spill[(k - r) * 128 : (k - r + 1) * 128, :]
                    )
                    rhs = ast
                for si, usrc in enumerate(srcs):
                    lt = usrc[k // kg][:, k % kg, :]
                    for dh in range(dh_n):
                        for mh in range(nmh):
                            nc.tensor.matmul(
                                psv[dh][mh],
                                lt[:, dh * 128 : (dh + 1) * 128],
                                rhs[:, mh * mw : (mh + 1) * mw],
                                start=(k == 0 and si == 0),
                                stop=(k == kc - 1 and si == len(srcs) - 1),
                            )

            # ---- Phase C: copy v.T to SBUF ----
            v_sb = [
                work.tile([128, s], F32, name="v_sb", tag="vsb") for _ in range(dh_n)
            ]
            for dh in range(dh_n):
                for mh in range(nmh):
                    nc.vector.tensor_copy(
                        v_sb[dh][:, mh * mw : (mh + 1) * mw], psv[dh][mh]
                    )

            # ---- Phase D: mm2 (fp32)  out.T = W @ v.T ; relu(+bias) ----
            pso = [
                [post.tile([128, mw], F32, name="pso", tag="post") for _ in range(nmh)]
                for _ in range(dh_n)
            ]
            for dho in range(dh_n):
                for kin in range(dh_n):
                    lt = wt_t[:, kin, dho * 128 : (dho + 1) * 128]
                    for mh in range(nmh):
                        nc.tensor.matmul(
                            pso[dho][mh],
                            lt,
                            v_sb[kin][:, mh * mw : (mh + 1) * mw],
                            start=(kin == 0),
                            stop=(kin == dh_n - 1),
                        )
            hT = [work.tile([128, s], F32, name="hT", tag="hT") for _ in range(dh_n)]
            for dho in range(dh_n):
                for mh in range(nmh):
                    sl_ = slice(mh * mw, (mh + 1) * mw)
                    nc.vector.tensor_mul(hT[dho][:, sl_], pso[dho][mh], dinv_bc[:, sl_])
                    nc.scalar.activation(
                        hT[dho][:, sl_],
                        hT[dho][:, sl_],
                        relu,
                        bias=bias_sb[:, 2 * l + dho : 2 * l + dho + 1],
                    )

            # ---- Phase E: transpose to natural, build next u slab, outputs ----
            is_out = l >= n_layers - 3  # layers 4,5,6 emit outputs 0,1,2
            save_skip = l <= 2
            h_nat = None
            if is_out or save_skip:
                h_nat = slabp.tile([128, mq, d], F32, name="h_nat", tag="hnat", bufs=1)
            if l == n_layers - 1:
                # reuse the (now idle) skip/us tags for the final-layer tiles
                h0s = slabp.tile([128, mq, d], F32, name="h0s", tag="skip")
                nc.sync.dma_start(
                    out=h0s, in_=h0s_dram[:, :].rearrange("(m p) d2 -> p m d2", p=128)
                )
                out3 = slabp.tile([128, mq, d], F32, name="out3", tag="us_hi", bufs=1)
            if l < n_layers - 1:
                us_hi = slabp.tile(
                    [128, mq, d], BF16, name="us_hi", tag="us_hi", bufs=1
                )
                if use_lo:
                    us_lo = slabp.tile(
                        [128, mq, d], BF16, name="us_lo", tag="us_lo", bufs=1
                    )

            for m in range(mq):
                tp = post.tile([128, d], F32, name="tp", tag="post")
                for dh in range(dh_n):
                    nc.tensor.transpose(
                        tp[:, dh * 128 : (dh + 1) * 128],
                        hT[dh][:, m * 128 : (m + 1) * 128],
                        ident,
                    )
                if l < n_layers - 1:
                    ufp = stage.tile([128, d], F32, name="ufp", tag="ufp")
                    dv = dinv_sb[:, m : m + 1]
                    if skip_sb is not None:
                        nc.vector.scalar_tensor_tensor(
                            out=ufp,
                            in0=tp,
                            scalar=dv,
                            in1=skip_sb[:, m, :],
                            op0=mybir.AluOpType.mult,
                            op1=mybir.AluOpType.add,
                        )
                    else:
                        nc.vector.tensor_scalar(
                            out=ufp,
                            in0=tp,
                            scalar1=dv,
                            scalar2=None,
                            op0=mybir.AluOpType.mult,
                        )
                    nc.vector.tensor_copy(us_hi[:, m, :], ufp)
                    if use_lo:
                        nc.vector.tensor_sub(us_lo[:, m, :], ufp, us_hi[:, m, :])
                if h_nat is not None:
                    nc.scalar.copy(h_nat[:, m, :], tp)
                if l == n_layers - 1:
                    nc.vector.tensor_add(out3[:, m, :], tp, h0s[:, m, :])

            # ---- Phase F: DMAs out + AllGather ----
            if save_skip:
                nc.sync.dma_start(
                    out=skip_dram[l].rearrange("(m p) d2 -> p m d2", p=128),
                    in_=h_nat,
                )
            if is_out:
                nc.sync.dma_start(
                    out=out_dram[l - (n_layers - 3)].rearrange(
                        "(m p) d2 -> p m d2", p=128
                    ),
                    in_=h_nat,
                )
            if l == n_layers - 1:
                nc.sync.dma_start(
                    out=out_dram[3].rearrange("(m p) d2 -> p m d2", p=128), in_=out3
                )
            if l < n_layers - 1:
                agv = ag_in.rearrange("(m p) td -> p m td", p=128)
                nc.sync.dma_start(out=agv[:, :, 0:d], in_=us_hi)
                if use_lo:
                    nc.sync.dma_start(out=agv[:, :, d : 2 * d], in_=us_lo)
                nc.gpsimd.collective_compute(
                    "AllGather",
                    mybir.AluOpType.bypass,
                    replica_groups=[list(range(c))],
                    ins=[ag_in.opt()],
                    outs=[ag_outs[rep * (n_layers - 1) + l].opt()],
                )

    nc.compile()
    return nc


def prep_inputs(g, h, W_down, b_down, W_bottom, b_bottom, W_up, b_up, c=C):
    """Host-side sharding + layout prep. Returns per-core input maps."""
    n = g.shape[0]
    s = n // c
    d = h.shape[1]
    g = np.asarray(g, np.float32)
    h = np.asarray(h, np.float32)
    deg = g.sum(axis=1) + 1.0
    dinv = (1.0 / np.sqrt(deg)).astype(np.float32)

    u0 = (h * dinv[:, None]).astype(np.float32)
    u0_hi = u0.astype(np.float32).astype(ml_bf16)
    u0_lo = (u0 - u0_hi.astype(np.float32)).astype(ml_bf16)
    u0_packed = np.concatenate(
        [np.asarray(u0_hi), np.asarray(u0_lo)], axis=1
    )  # [n, 2d] bf16

    Ws = [W_down[0], W_down[1], W_down[2], W_bottom, W_up[0], W_up[1], W_up[2]]
    bs = [b_down[0], b_down[1], b_down[2], b_bottom, b_up[0], b_up[1], b_up[2]]
    wt = np.stack([np.ascontiguousarray(np.asarray(W, np.float32).T) for W in Ws])
    nl = len(Ws)
    bias_t = np.zeros((128, 2 * nl), np.float32)
    for li, b in enumerate(bs):
        b = np.asarray(b, np.float32)
        for dh in range(d // 128):
            bias_t[:, 2 * li + dh] = b[dh * 128 : (dh + 1) * 128]

    in_maps = []
    for ci in range(c):
        sl = slice(ci * s, (ci + 1) * s)
        a_slab = np.ascontiguousarray(g[:, sl])
        idx = np.arange(s)
        a_slab[ci * s + idx, idx] += 1.0  # fold self-loops into the slab
        dinv_slab = dinv[sl].reshape(s // 128, 128).T.copy()  # [128, mq]
        dinv_bcast = np.broadcast_to(dinv[sl][None, :], (128, s)).copy()
        in_maps.append(
            dict(
                a_slab=a_slab,
                u0=u0_packed,
                h0_slab=np.ascontiguousarray(h[sl]),
                dinv_slab=dinv_slab,
                dinv_bcast=dinv_bcast,
                wt=wt,
                bias_t=bias_t,
            )
        )
    return in_maps


try:
    import ml_dtypes

    ml_bf16 = ml_dtypes.bfloat16
except ImportError:  # pragma: no cover
    import jax.numpy as jnp

    ml_bf16 = jnp.bfloat16

_NC_CACHE = {}


def kernel(g, h, W_down, b_down, W_bottom, b_bottom, W_up, b_up):
    key = "full"
    if key not in _NC_CACHE:
        _NC_CACHE[key] = build_nc()
    nc = _NC_CACHE[key]
    in_maps = prep_inputs(g, h, W_down, b_down, W_bottom, b_bottom, W_up, b_up)
    res = run_bass_kernel_spmd(nc, in_maps, list(range(C)))
    outs = [np.asarray(r["out"]).reshape(4, S, D) for r in res.results]
    full = np.concatenate(outs, axis=1)  # [4, N, D]
    return full.astype(np.float32)


if __name__ == "__main__":
    import reference

    inputs = reference.setup_inputs()
    inputs = {k: np.asarray(v) for k, v in inputs.items()}
    out = kernel(**inputs)
    exp = np.asarray(reference.reference(**reference.setup_inputs()))
    err = np.abs(out - exp).max() / (np.abs(exp).max() + 1e-30)
    rel = np.linalg.norm(out - exp) / (np.linalg.norm(exp) + 1e-30)
    print("max-scaled err:", err, "rel l2:", rel)
